# revision 3
# baseline (speedup 1.0000x reference)
"""Packed causal GQA attention (B=4 x S=1024, H=32, KVH=8, D=DV=128, fp32)
for 8 Trainium2 NeuronCores.

Sharding: tensor-parallel over KV heads. Core c owns kv head c and its GQA
group of 4 query heads (4c..4c+3). No cross-core communication.

Per-core pipeline (16 units of (b, h)):
  - QK: S^T[k, q] = K^T.T @ Q^T on PE (fp16, fp32 PSUM), causal col ranges.
  - P^T = Exp(SCALE*S^T) on ACT into a dense SBUF tile pt[128, kb, 1024];
    the strictly-upper triangle of each diagonal block is zeroed by a
    gpsimd affine_select.
  - PV with fused denominator: rhs = [V | 1] (129 cols); for each q-block
    qb, out_psum[q, 0:128] = sum_kb P^T_kb(qb).T @ V_kb and
    out_psum[q, 128] = softmax denominator l[q], accumulated over kb in a
    single-bank PSUM tile. This kills the separate ones-matmul (1/3 of
    all PE columns in the old scheme) and yields the output in natural
    [q, dv] layout (no host-side transpose).
  - normalize: rinv[q] = 1/l on DVE, out = psum * rinv (per-partition
    tensor_scalar), fp16 store.
"""

from collections import deque

import numpy as np

import concourse.bacc as bacc
import concourse.tile as tile
from concourse import mybir, bass_utils

T = 4096          # packed tokens
SEQ = 1024        # per-sequence length
B = T // SEQ      # 4 sequences
H = 32            # query heads (total)
KVH = 8           # kv heads (total)
D = 128           # head size
DV = 128          # value head size
NCORES = 8
HPC = H // NCORES         # 4 query heads per core
NB = SEQ // 128           # 8 k-blocks per sequence
SCALE = 0.08838834764831845

F16 = mybir.dt.float16
F32 = mybir.dt.float32

_BUILD_CACHE = {}


def _build_nc():
    nc = bacc.Bacc("TRN2", target_bir_lowering=False, debug=False,
                   num_devices=NCORES)
    # host-pretransposed, fp16: qT[h*128+d, t], kT[d, t]; v[t, dv]
    qt_dram = nc.dram_tensor("qT", [HPC * D, T], F16, kind="ExternalInput").ap()
    kt_dram = nc.dram_tensor("kT", [D, T], F16, kind="ExternalInput").ap()
    v_dram = nc.dram_tensor("v", [T, DV], F16, kind="ExternalInput").ap()
    # natural layout output: out[t, h*DV + dv] for this core's 4 heads
    out_dram = nc.dram_tensor("out", [T, HPC * DV], F16,
                              kind="ExternalOutput").ap()

    with tile.TileContext(nc) as tc:
        with tc.tile_pool(name="kv", bufs=2) as kv_pool, \
             tc.tile_pool(name="qts", bufs=8) as qt_pool, \
             tc.tile_pool(name="pt", bufs=2) as pt_pool, \
             tc.tile_pool(name="work", bufs=2) as work, \
             tc.tile_pool(name="pp_s", bufs=3, space="PSUM") as pp_s, \
             tc.tile_pool(name="pp_v", bufs=2, space="PSUM") as pp_v:

            per_b = {}   # b -> (kt, vext, [qt0..qt3])

            def emit_loads(b):
                cols = slice(b * SEQ, (b + 1) * SEQ)
                rows = slice(b * SEQ, (b + 1) * SEQ)
                kt = kv_pool.tile([128, NB, 128], F16, tag="kt")
                nc.sync.dma_start(
                    kt[:], kt_dram[:, cols].rearrange("d (nb t) -> d nb t", t=128))
                qts = []
                for h in range(HPC):
                    qt = qt_pool.tile([128, NB, 128], F16, tag="qt")
                    nc.sync.dma_start(
                        qt[:],
                        qt_dram[h * D:(h + 1) * D, cols].rearrange(
                            "d (nb t) -> d nb t", t=128))
                    qts.append(qt)
                vext = kv_pool.tile([128, NB, DV + 1], F16, tag="v")
                nc.sync.dma_start(
                    vext[:, :, 0:DV],
                    v_dram[rows, :].rearrange("(nb p) d -> p nb d", p=128))
                nc.vector.memset(vext[:, :, DV:DV + 1], 1.0)
                per_b[b] = (kt, vext, qts)

            def emit_unit(b, h):
                kt, vext, qts = per_b[b]
                qt = qts[h]
                pt = pt_pool.tile([128, NB, SEQ], F16, tag="pt")
                out_sb = work.tile([128, NB, DV], F16, tag="out_sb")

                def emit_qk(kb):
                    """QK matmuls + exp + causal mask for k-block kb."""
                    ps = pp_s.tile([128, 1024], F32, tag="ps")
                    for qc in range(kb // 4, 2):
                        qs = max(128 * kb, 512 * qc)
                        qe = 512 * (qc + 1)
                        nc.tensor.matmul(
                            ps[:, qs:qe],
                            kt[:, kb, :],
                            qt[:, qs // 128:qe // 128, :],
                            start=True, stop=True, skip_group_check=True)
                    nc.scalar.activation(
                        pt[:, kb, 128 * kb:], ps[:, 128 * kb:],
                        mybir.ActivationFunctionType.Exp, scale=SCALE)
                    # zero strictly-upper triangle of the diagonal block
                    nc.gpsimd.affine_select(
                        out=pt[:, kb, 128 * kb:128 * (kb + 1)],
                        in_=pt[:, kb, 128 * kb:128 * (kb + 1)],
                        compare_op=mybir.AluOpType.is_ge,
                        fill=0.0, base=0,
                        pattern=[[1, 128]], channel_multiplier=-1)

                def emit_pv(qb):
                    """PV + fused denominator for q-block qb, normalize."""
                    pv = pp_v.tile([128, DV + 1], F32, tag="pv")
                    for kb in range(qb + 1):
                        nc.tensor.matmul(
                            pv[:],
                            pt[:, kb, 128 * qb:128 * (qb + 1)],
                            vext[:, kb, :],
                            start=(kb == 0), stop=(kb == qb),
                            skip_group_check=True)
                    rinv = work.tile([128, 1], F32, tag="rinv", bufs=3)
                    nc.vector.reciprocal_approx_fast(rinv[:], pv[:, DV:DV + 1])
                    nc.vector.tensor_scalar(
                        out_sb[:, qb, :], pv[:, 0:DV], rinv[:], None,
                        mybir.AluOpType.mult)

                # stagger: QK(kb+1) is emitted before PV(qb=kb) so the PE
                # never waits in-order on the ACT exp it just triggered
                emit_qk(0)
                for kb in range(1, NB):
                    emit_qk(kb)
                    emit_pv(kb - 1)
                emit_pv(NB - 1)

                nc.sync.dma_start(
                    out_dram[b * SEQ:(b + 1) * SEQ,
                             h * DV:(h + 1) * DV].rearrange(
                        "(qb p) d -> p qb d", p=128),
                    out_sb[:])

            units = [(b, h) for b in range(B) for h in range(HPC)]
            for b, h in units:
                if h == 0:
                    emit_loads(b)
                emit_unit(b, h)

    nc.compile()
    return nc


def run_sharded(query, key, value, trace=False):
    """Shard over 8 cores, run the bass kernel, unshard. Returns
    (out [T, H*DV] fp32, BassKernelResults)."""
    query = np.asarray(query, dtype=np.float32)
    key = np.asarray(key, dtype=np.float32)
    value = np.asarray(value, dtype=np.float32)

    if "nc" not in _BUILD_CACHE:
        _BUILD_CACHE["nc"] = _build_nc()
    nc = _BUILD_CACHE["nc"]

    # host layout glue: cast to fp16, then transpose to [d, t]
    qT = np.ascontiguousarray(query.astype(np.float16).T)   # [H*D, T]
    kT = np.ascontiguousarray(key.astype(np.float16).T)     # [KVH*D, T]
    v16 = np.ascontiguousarray(value.astype(np.float16))    # [T, KVH*DV]

    in_maps = []
    for c in range(NCORES):
        in_maps.append({
            "qT": np.ascontiguousarray(qT[c * HPC * D:(c + 1) * HPC * D]),
            "kT": np.ascontiguousarray(kT[c * D:(c + 1) * D]),
            "v": np.ascontiguousarray(v16[:, c * DV:(c + 1) * DV]),
        })

    res = bass_utils.run_bass_kernel_spmd(
        nc, in_maps, core_ids=list(range(NCORES)), trace=trace)

    out = np.concatenate(
        [res.results[c]["out"].astype(np.float32) for c in range(NCORES)],
        axis=1)
    return out, res


def kernel(query, key, value, seq_len=1024, **_unused):
    assert int(seq_len) == SEQ, f"kernel hardcodes seq_len={SEQ}, got {seq_len}"
    out, _ = run_sharded(query, key, value, trace=False)
    return out
